# revision 10
# baseline (speedup 1.0000x reference)
"""FP8-style block-dequant linear: y = x @ (weight * block_scales).T

Full-input contract: kernel(x, weight, weight_scale_inv) -> y [32, 18432] f32.

Strategy (column-parallel over 8 NeuronCores):
  - Shard weight rows (out_features) across cores: each core owns
    O_LOC = 18432/8 = 2304 rows -> computes y[:, c*2304:(c+1)*2304].
  - Host-side layout prep (pure data movement): per-core transposed
    weight W^T [7168, 2304] so the contraction dim lands on SBUF
    partitions with large contiguous DMA lines; x packed into its SBUF
    tile layout; scales replicated across partitions.
  - On-device per core (exact fp32, memory-bound at ~330 GB/s/core):
      * stream W^T k-tiles from HBM (2 tiles per DMA, 2.4 MB each)
      * dequant-scale on DVE (block scale per 128-column group)
      * accumulate into PSUM with x^T tiles [128, 32] stationary.
        fp32 matmul is 4 cyc/row, so 4 independent M=32 matmuls run
        concurrently in separate PE column groups (tile_position) --
        measured 3.7x, bringing PE under the DMA roofline.
"""

import numpy as np

M = 32
I = 7168
O = 18432
NCORES = 8
O_LOC = O // NCORES  # 2304
BLK = 128
IB = I // BLK  # 56 k-tiles
OBL = O_LOC // BLK  # 18 block-columns per core
GRP = 2  # k-tiles per weight DMA
NTAIL = O_LOC - 4 * 512  # 256

_CACHE = {}


def _build_nc(iters=1):
    import concourse.mybir as mybir
    from concourse import bacc
    from concourse.tile import TileContext

    f32 = mybir.dt.float32
    nc = bacc.Bacc()
    wt = nc.declare_dram_parameter("wt", [I, O_LOC], f32, isOutput=False)
    xp = nc.declare_dram_parameter("xp", [BLK, IB * M], f32, isOutput=False)
    ss = nc.declare_dram_parameter("ss", [BLK, IB * OBL], f32, isOutput=False)
    y = nc.declare_dram_parameter("y", [M, O_LOC], f32, isOutput=True)

    wt_g = wt[:, :].rearrange("(g t p) o -> g p t o", t=GRP, p=BLK)

    with TileContext(nc) as tc:
        with (
            tc.tile_pool(name="consts", bufs=1) as consts,
            tc.tile_pool(name="wp", bufs=3) as wp,
            tc.tile_pool(name="wsp", bufs=3) as wsp,
            tc.tile_pool(name="pp", bufs=1, space="PSUM") as pp,
            tc.tile_pool(name="op", bufs=2) as op,
        ):
            xs = consts.tile([BLK, IB * M], f32)
            nc.sync.dma_start(out=xs, in_=xp[:, :])
            sc = consts.tile([BLK, IB * OBL], f32)
            nc.sync.dma_start(out=sc, in_=ss[:, :])

            import contextlib

            loop_ctx = (
                tc.For_i(0, iters, 1, hint_engines=(mybir.EngineType.PE,))
                if iters > 1
                else contextlib.nullcontext()
            )
            with loop_ctx:
                psa = pp.tile([BLK, 512], f32)
                psb = pp.tile([M, NTAIL], f32)

                for g in range(IB // GRP):
                    w = wp.tile([BLK, GRP * O_LOC], f32)
                    nc.sync.dma_start(
                        out=w.rearrange("p (t o) -> p t o", t=GRP), in_=wt_g[g]
                    )
                    ws = wsp.tile([BLK, GRP * O_LOC], f32)
                    nc.vector.tensor_mul(
                        out=ws.rearrange("p (b oc) -> p b oc", oc=BLK),
                        in0=w.rearrange("p (b oc) -> p b oc", oc=BLK),
                        in1=sc[
                            :, g * GRP * OBL : (g + 1) * GRP * OBL
                        ].broadcast_to((BLK, GRP * OBL, BLK)),
                    )
                    for t in range(GRP):
                        ib = g * GRP + t
                        lhsT = xs[:, ib * M : (ib + 1) * M]
                        first, last = ib == 0, ib == IB - 1
                        for j in range(4):
                            nc.tensor.matmul(
                                psa[32 * j : 32 * (j + 1), :],
                                lhsT,
                                ws[:, t * O_LOC + j * 512 : t * O_LOC + (j + 1) * 512],
                                start=first,
                                stop=last,
                                tile_position=(0, 32 * j),
                                skip_group_check=True,
                            )
                        nc.tensor.matmul(
                            psb,
                            lhsT,
                            ws[:, t * O_LOC + 2048 : t * O_LOC + O_LOC],
                            start=first,
                            stop=last,
                            tile_position=(0, 0),
                            skip_group_check=True,
                        )

                ysb = op.tile([M, O_LOC], f32)
                for j in range(4):
                    nc.vector.tensor_copy(
                        out=ysb[:, j * 512 : (j + 1) * 512],
                        in_=psa[32 * j : 32 * (j + 1), :],
                    )
                nc.vector.tensor_copy(out=ysb[:, 2048:O_LOC], in_=psb)
                nc.sync.dma_start(out=y[:, :], in_=ysb)
    nc.compile()
    return nc


def get_nc(iters=1):
    key = ("nc", iters)
    if key not in _CACHE:
        _CACHE[key] = _build_nc(iters)
    return _CACHE[key]


def make_in_maps(x, weight, weight_scale_inv):
    """Host-side shard + layout prep (pure data movement, no arithmetic)."""
    x = np.ascontiguousarray(x, dtype=np.float32)
    weight = np.ascontiguousarray(weight, dtype=np.float32)
    s = np.ascontiguousarray(weight_scale_inv, dtype=np.float32)

    # x packed: xp[p, ib*M + m] = x[m, ib*BLK + p]
    xp = np.ascontiguousarray(
        x.reshape(M, IB, BLK).transpose(2, 1, 0).reshape(BLK, IB * M)
    )

    in_maps = []
    for c in range(NCORES):
        w_c = weight[c * O_LOC : (c + 1) * O_LOC, :]  # [O_LOC, I]
        wt_c = np.ascontiguousarray(w_c.T)  # [I, O_LOC]
        s_c = s[c * OBL : (c + 1) * OBL, :]  # [OBL, IB]
        ss_flat = np.ascontiguousarray(s_c.T).reshape(1, IB * OBL)
        ss_c = np.ascontiguousarray(np.broadcast_to(ss_flat, (BLK, IB * OBL)))
        in_maps.append({"wt": wt_c, "xp": xp, "ss": ss_c})
    return in_maps


def kernel(x, weight, weight_scale_inv):
    from concourse.bass_utils import run_bass_kernel_spmd

    nc = get_nc()
    in_maps = make_in_maps(x, weight, weight_scale_inv)
    res = run_bass_kernel_spmd(nc, in_maps, list(range(NCORES)))
    outs = [res.results[c]["y"] for c in range(NCORES)]
    return np.ascontiguousarray(np.concatenate(outs, axis=1), dtype=np.float32)
